# revision 1
# baseline (speedup 1.0000x reference)
"""MoE (top-2 of 8 experts) Trainium2 kernel, 8-core data-parallel over tokens.

Problem shapes (hardcoded): x [4, 2048, 512] f32, Wg [512, 8], W1 [8, 512, 1024],
b1 [8, 1024], W2 [8, 1024, 512], b2 [8, 512].  T = 8192 tokens, top-2 routing.

Strategy: shard tokens across the 8 cores (1024/core); replicate router and
expert weights (weights cast to bf16 host-side).  Fully on device, per core:
  1. xT via PE transpose; fp32 router -> softmax -> top-2 (DVE max8).
  2. Per 128-token tile: within-tile token rank per expert via a
     triangular-ones matmul prefix-sum; slot = e*CAP + tile*CAPT + rank
     (per-tile local capacity CAPT=48, so tiles dispatch independently);
     (token_id, gate) pairs and bf16 x rows scattered to the slot via
     indirect DMA, interleaved with the next tiles' router work.
  3. Per expert: load its <=CAP staged rows (regular DMA), PE-transpose,
     bf16 GEMM1 -> fused gelu_tanh(+b1) -> bf16 GEMM2, multiply by gate,
     write gated y rows slot-ordered (regular parallel DMAs, no WAW chain).
  4. Final combine per token tile: indirect-gather the token's two y rows by
     the saved slot ids, add, write the output contiguously token-major.
Padded slots carry gate=0 and are simply never gathered by any token.
"""

from contextlib import ExitStack

import numpy as np
import ml_dtypes

import concourse.bass as bass
import concourse.tile as tile
from concourse import bacc, mybir
from concourse.bass import IndirectOffsetOnAxis
from concourse.bass_utils import run_bass_kernel_spmd
from concourse.masks import make_identity

P = 128
N_CORES = 8
B, S, D, H, O, E = 4, 2048, 512, 1024, 512, 8
T = B * S                    # 8192
TC = T // N_CORES            # 1024 tokens per core
DC = D // P                  # 4 D-chunks
HC = H // P                  # 8 H-chunks
NT = TC // P                 # 8 token tiles of 128
CAP = 384                    # per-expert token capacity (3 tiles of 128)
NS = CAP // P                # 3 slot tiles per expert
CAPT = CAP // NT             # 48: per-(tile, expert) local capacity

MM_DT = mybir.dt.bfloat16
NP_MM_DT = ml_dtypes.bfloat16
F32 = mybir.dt.float32
I32 = mybir.dt.int32
AF = mybir.ActivationFunctionType
ALU = mybir.AluOpType


def build_nc(has_b1: bool, has_b2: bool) -> bass.Bass:
    nc = bacc.Bacc()
    x_d = nc.declare_dram_parameter("x", [TC, D], F32, isOutput=False)
    wg_d = nc.declare_dram_parameter("wg", [D, E], F32, isOutput=False)
    w1_d = nc.declare_dram_parameter("w1", [E, D, H], MM_DT, isOutput=False)
    w2_d = nc.declare_dram_parameter("w2", [E, H, O], MM_DT, isOutput=False)
    if has_b1:
        b1_d = nc.declare_dram_parameter("b1", [E, H], F32, isOutput=False)
    if has_b2:
        b2_d = nc.declare_dram_parameter("b2", [E, O], F32, isOutput=False)
    out_d = nc.declare_dram_parameter("out", [TC, O], F32, isOutput=True)

    xg_d = nc.dram_tensor("xg", [E * CAP, D], MM_DT)
    y_d = nc.dram_tensor("yd", [E * CAP, O], F32)

    with ExitStack() as ctx:
        tc = ctx.enter_context(tile.TileContext(nc))
        singles = ctx.enter_context(tc.tile_pool(name="singles", bufs=1))
        xload = ctx.enter_context(tc.tile_pool(name="xload", bufs=3))
        wpool = ctx.enter_context(tc.tile_pool(name="wpool", bufs=4))
        hpool = ctx.enter_context(tc.tile_pool(name="hpool", bufs=2))
        tmp = ctx.enter_context(tc.tile_pool(name="tmp", bufs=4))
        psum_t = ctx.enter_context(tc.tile_pool(name="psum_t", bufs=2, space="PSUM"))
        psum_r = ctx.enter_context(tc.tile_pool(name="psum_r", bufs=1, space="PSUM"))
        psum_rk = ctx.enter_context(tc.tile_pool(name="psum_rk", bufs=1, space="PSUM"))
        psum_h = ctx.enter_context(tc.tile_pool(name="psum_h", bufs=2, space="PSUM"))
        psum_y = ctx.enter_context(tc.tile_pool(name="psum_y", bufs=2, space="PSUM"))

        ident = singles.tile([P, P], F32)
        make_identity(nc, ident)
        ident16 = singles.tile([P, P], MM_DT)
        nc.vector.tensor_copy(ident16, ident)

        # inclusive lower-triangular ones: tril[q, p] = 1.0 iff q <= p
        tril = singles.tile([P, P], F32)
        nc.gpsimd.memset(tril, 0.0)
        nc.gpsimd.affine_select(
            out=tril, in_=tril, compare_op=ALU.is_gt, fill=1.0,
            base=0, pattern=[[-1, P]], channel_multiplier=1,
        )

        wg_sb = singles.tile([P, DC, E], F32)
        nc.sync.dma_start(wg_sb, wg_d[:].rearrange("(c p) e -> p c e", p=P))
        if has_b1:
            b1_sb = singles.tile([P, HC, E], F32)
            with nc.allow_non_contiguous_dma(reason="tiny one-time b1 load"):
                nc.sync.dma_start(b1_sb, b1_d[:].rearrange("e (c p) -> p c e", p=P))
        if has_b2:
            b2_sb = singles.tile([P, E, O], F32)
            b2_ap = b2_d[:]
            b2_bcast = bass.AP(
                tensor=b2_ap.tensor, offset=b2_ap.offset, ap=[[0, P], *b2_ap.ap]
            )
            nc.sync.dma_start(b2_sb, b2_bcast)

        # iotas: per-(tile, expert) slot bases and token ids
        iota_base_i = singles.tile([P, NT, E], I32)
        nc.gpsimd.iota(
            iota_base_i, pattern=[[CAPT, NT], [CAP, E]], base=0, channel_multiplier=0
        )
        iota_base = singles.tile([P, NT, E], F32)
        nc.vector.tensor_copy(iota_base, iota_base_i)
        iota_tok_i = singles.tile([P, NT], I32)
        nc.gpsimd.iota(iota_tok_i, pattern=[[P, NT]], base=0, channel_multiplier=1)
        iota_tok = singles.tile([P, NT], F32)
        nc.vector.tensor_copy(iota_tok, iota_tok_i)

        zeros16 = singles.tile([P, D], MM_DT)
        nc.vector.memset(zeros16, 0.0)
        z_ap = zeros16[:]
        z_src = bass.AP(
            tensor=z_ap.tensor, offset=z_ap.offset,
            ap=[z_ap.ap[0], [0, E * CAP // P], *z_ap.ap[1:]],
        )
        nc.sync.dma_start(xg_d[:].rearrange("(p a) d -> p a d", p=P), z_src)

        xT32 = singles.tile([P, DC, TC], F32)
        x16_all = singles.tile([P, NT, D], MM_DT)
        slotk_all = singles.tile([P, NT, 2], I32)
        gates_all = singles.tile([P, NT, 2], F32)

        # ---- transpose x into xT (fp32, for router) + bf16 copy in SBUF ----
        for tt in range(NT):
            xr = xload.tile([P, D], F32)
            nc.sync.dma_start(xr, x_d[:][tt * P:(tt + 1) * P, :])
            nc.vector.tensor_copy(x16_all[:, tt, :], xr)
            for dc in range(DC):
                pt = psum_t.tile([P, P], F32, tag="pt")
                nc.tensor.transpose(pt, xr[:, dc * P:(dc + 1) * P], ident)
                nc.vector.tensor_copy(xT32[:, dc, tt * P:(tt + 1) * P], pt)

        # ---- per tile: router, top-2, local rank, slots, dispatch scatters ----
        for tt in range(NT):
            pr = psum_r.tile([P, E], F32, tag="pr")
            for dc in range(DC):
                nc.tensor.matmul(
                    pr, lhsT=xT32[:, dc, tt * P:(tt + 1) * P], rhs=wg_sb[:, dc, :],
                    start=(dc == 0), stop=(dc == DC - 1),
                )
            # top-2 selection runs on unnormalized exp(logits); the softmax
            # denominator only scales the two gate values at pair-copy time,
            # keeping reciprocal off the selection critical path.
            ex = tmp.tile([P, E], F32, tag="ex")
            s = tmp.tile([P, 1], F32, tag="s")
            nc.scalar.activation(out=ex, in_=pr, func=AF.Exp, accum_out=s)
            rec = tmp.tile([P, 1], F32, tag="rec")
            nc.vector.reciprocal(rec, s)
            top8 = tmp.tile([P, 8], F32, tag="top8")
            nc.vector.max(out=top8, in_=ex)
            mask = tmp.tile([P, E], F32, tag="mask")
            nc.vector.tensor_scalar(
                out=mask, in0=ex, scalar1=top8[:, 1:2], scalar2=None, op0=ALU.is_ge
            )
            # within-tile inclusive rank via triangular-ones matmul; dedicated
            # single-bank pool so pr(t+1) no longer waits on prk(t)
            prk = psum_rk.tile([P, E], F32, tag="prk")
            nc.tensor.matmul(prk, lhsT=tril, rhs=mask, start=True, stop=True)

            slots = tmp.tile([P, E], F32, tag="slots")
            nc.vector.tensor_sub(slots, prk, mask)  # exclusive rank
            nc.vector.tensor_add(slots, slots, iota_base[:, tt, :])
            oh1 = tmp.tile([P, E], F32, tag="oh1")
            nc.vector.tensor_scalar(
                out=oh1, in0=ex, scalar1=top8[:, 0:1], scalar2=None, op0=ALU.is_equal
            )
            sel = tmp.tile([P, E], F32, tag="sel")
            slotk_f = tmp.tile([P, 2], F32, tag="slotk_f")
            nc.vector.tensor_mul(sel, oh1, slots)
            nc.vector.reduce_sum(slotk_f[:, 0:1], sel, axis=mybir.AxisListType.X)
            nc.vector.tensor_sub(sel, mask, oh1)  # top-2 one-hot
            nc.vector.tensor_mul(sel, sel, slots)
            nc.vector.reduce_sum(slotk_f[:, 1:2], sel, axis=mybir.AxisListType.X)
            slotk_i = slotk_all[:, tt, :]
            nc.vector.tensor_copy(slotk_i, slotk_f)

            nc.vector.tensor_scalar_mul(gates_all[:, tt, :], top8[:, 0:2], rec)
            for k in range(2):
                nc.gpsimd.indirect_dma_start(
                    out=xg_d[:],
                    out_offset=IndirectOffsetOnAxis(
                        ap=slotk_i[:, k:k + 1], axis=0
                    ),
                    in_=x16_all[:, tt, :],
                    in_offset=None,
                    bounds_check=E * CAP - 1,
                    oob_is_err=False,
                )

        # ---- staging: load all experts' rows (parallel HWDGE) + transpose ----
        xTg_all = singles.tile([P, DC, E * CAP], MM_DT)
        for e in range(E):
            for sl in range(NS):
                xg = xload.tile([P, D], MM_DT, tag="xg")
                nc.sync.dma_start(
                    xg, xg_d[:][e * CAP + sl * P:e * CAP + (sl + 1) * P, :]
                )
                for dc in range(DC):
                    pt16 = psum_t.tile([P, P], MM_DT, tag="pt")
                    nc.tensor.transpose(pt16, xg[:, dc * P:(dc + 1) * P], ident16)
                    nc.vector.tensor_copy(
                        xTg_all[:, dc, e * CAP + sl * P:e * CAP + (sl + 1) * P],
                        pt16,
                    )

        # ---- compute phase: per-expert MLP + gated scatter-add ----
        for e in range(E):
            w1_sb = wpool.tile([P, DC, H], MM_DT, tag="w1")
            nc.sync.dma_start(w1_sb, w1_d[:][e].rearrange("(c p) h -> p c h", p=P))
            w2_sb = wpool.tile([P, HC, O], MM_DT, tag="w2")
            nc.sync.dma_start(w2_sb, w2_d[:][e].rearrange("(c p) o -> p c o", p=P))

            h_sb = hpool.tile([P, HC, CAP], MM_DT, tag="h")
            for hc in range(HC):
                ph = psum_h.tile([P, CAP], F32)
                for dc in range(DC):
                    nc.tensor.matmul(
                        ph, lhsT=w1_sb[:, dc, hc * P:(hc + 1) * P],
                        rhs=xTg_all[:, dc, e * CAP:(e + 1) * CAP],
                        start=(dc == 0), stop=(dc == DC - 1),
                    )
                bias_ap = b1_sb[:, hc, e:e + 1] if has_b1 else 0.0
                nc.scalar.activation(
                    out=h_sb[:, hc, :], in_=ph, func=AF.Gelu_apprx_tanh, bias=bias_ap
                )

            for sl in range(NS):
                py = psum_y.tile([P, O], F32)
                for hc in range(HC):
                    nc.tensor.matmul(
                        py, lhsT=h_sb[:, hc, sl * P:(sl + 1) * P], rhs=w2_sb[:, hc, :],
                        start=(hc == 0), stop=(hc == HC - 1),
                    )
                yg = tmp.tile([P, O], F32, tag="yg")
                if has_b2:
                    nc.vector.tensor_add(yg, py, b2_sb[:, e, :])
                else:
                    nc.vector.tensor_copy(yg, py)
                nc.sync.dma_start(
                    y_d[:][e * CAP + sl * P:e * CAP + (sl + 1) * P, :], yg
                )

        # ---- final combine: per token, gather its two gated y rows and add ----
        for tt in range(NT):
            y1 = xload.tile([P, O], F32, tag="y1")
            nc.gpsimd.indirect_dma_start(
                out=y1,
                out_offset=None,
                in_=y_d[:],
                in_offset=IndirectOffsetOnAxis(
                    ap=slotk_all[:, tt, 0:1], axis=0
                ),
                bounds_check=E * CAP - 1,
                oob_is_err=False,
            )
            y2 = xload.tile([P, O], F32, tag="y2")
            nc.gpsimd.indirect_dma_start(
                out=y2,
                out_offset=None,
                in_=y_d[:],
                in_offset=IndirectOffsetOnAxis(
                    ap=slotk_all[:, tt, 1:2], axis=0
                ),
                bounds_check=E * CAP - 1,
                oob_is_err=False,
            )
            nc.vector.tensor_scalar_mul(y1, y1, gates_all[:, tt, 0:1])
            nc.vector.tensor_scalar_mul(y2, y2, gates_all[:, tt, 1:2])
            nc.vector.tensor_add(y1, y1, y2)
            nc.sync.dma_start(out_d[:][tt * P:(tt + 1) * P, :], y1)

    nc.finalize()
    return nc


_NC_CACHE: dict = {}


def _get_nc(has_b1: bool, has_b2: bool) -> bass.Bass:
    key = (has_b1, has_b2)
    if key not in _NC_CACHE:
        _NC_CACHE[key] = build_nc(has_b1, has_b2)
    return _NC_CACHE[key]


def kernel(x, Wg, W1, b1, W2, b2, _trace=False, _tmpdir=None):
    x = np.ascontiguousarray(np.asarray(x, dtype=np.float32))
    Wg = np.ascontiguousarray(np.asarray(Wg, dtype=np.float32))
    W1 = np.asarray(W1, dtype=np.float32)
    b1 = np.asarray(b1, dtype=np.float32)
    W2 = np.asarray(W2, dtype=np.float32)
    b2 = np.asarray(b2, dtype=np.float32)

    has_b1 = bool(np.any(b1))
    has_b2 = bool(np.any(b2))
    nc = _get_nc(has_b1, has_b2)

    xm = x.reshape(T, D)
    w1_bf = np.ascontiguousarray(W1.astype(NP_MM_DT))
    w2_bf = np.ascontiguousarray(W2.astype(NP_MM_DT))

    base = {"wg": Wg, "w1": w1_bf, "w2": w2_bf}
    if has_b1:
        base["b1"] = np.ascontiguousarray(b1)
    if has_b2:
        base["b2"] = np.ascontiguousarray(b2)

    in_maps = [
        {**base, "x": np.ascontiguousarray(xm[c * TC:(c + 1) * TC])}
        for c in range(N_CORES)
    ]
    res = run_bass_kernel_spmd(
        nc, in_maps, core_ids=list(range(N_CORES)), trace=_trace, tmpdir=_tmpdir
    )
    out = np.concatenate([res.results[c]["out"] for c in range(N_CORES)], axis=0)
    if _trace:
        kernel._last_result = res
    return out.reshape(B, S, O).astype(np.float32)

